# revision 1
# baseline (speedup 1.0000x reference)
"""Trainium2 Bass kernel for nn_MultiHeadAttention_47330539602717.

Math (per batch b, head h; q/k projections are dead code in the reference):
    vpT   = Wv^T @ v_b^T                        (1024, 4096)  [no bias]
    A^T_h = i_h @ vpT_h + (i_h @ bv_h)[:,None]  (128 q, 4096 s)
    P1    = exp(A^T) / colsum(exp(A^T))         softmax over q (partition dim)
    exp2  = exp(9 * P1)
    Pfold[l, qr] = sum_j exp2[l, qr + 128 j]    (torch raw .view fold)
    w[q]  = sum_l Pfold[l, q] / Z2[l],  Z2[l] = sum_qr Pfold[l, qr]
    x_h   = w @ i_h                             (64,)
    out_b = concat_h(x_h) @ Wo + bo             (1, 1024)

Sharding: data-parallel over batch. Core c handles batch b=c (all 16 heads).
Engine plan per core: PE does the big GEMM (f32r full-rate), per-head A^T,
S1 column-sums (indicator matmuls), and S1-replication; ACT does exp1 (with
the bv bias folded per-partition), exp2, and 1/S1 via Ln+Exp(-x); DVE does
the softmax1 divide-multiply, the j-fold adds, and PSUM->SBUF moves.
"""

import sys

import numpy as np

sys.path.insert(0, "/opt/trn_rl_repo")

from contextlib import ExitStack

import concourse.bacc as bacc
import concourse.tile as tile
from concourse import mybir
from concourse.bass_utils import run_bass_kernel_spmd

F32 = mybir.dt.float32
F32R = mybir.dt.float32r
BF16 = mybir.dt.bfloat16
EXP = mybir.ActivationFunctionType.Exp
LN = mybir.ActivationFunctionType.Ln
AX = mybir.AxisListType.X

B, LQ, S, D, H = 8, 128, 4096, 1024, 16
DK = D // H          # 64
KD = 8               # k blocks of 128 over D
SCP = 4              # outer s chunks (1024 cols each)
SUB = 2              # 512-col substeps per chunk
NCH = 512
SMOOTH = 9.0


def r(ap):
    return ap.bitcast(F32R)


def build_program():
    nc = bacc.Bacc("TRN2", target_bir_lowering=False, debug=False,
                   num_devices=8)

    vT_d = nc.dram_tensor("vT", [D, S], F32, kind="ExternalInput").ap()
    iT_d = nc.dram_tensor("iT", [128, 8, 128], F32, kind="ExternalInput").ap()
    iN_d = nc.dram_tensor("iN", [128, H, DK], BF16, kind="ExternalInput").ap()
    cb_d = nc.dram_tensor("cb", [128, H], F32, kind="ExternalInput").ap()
    Wv_d = nc.dram_tensor("Wv", [D, D], F32, kind="ExternalInput").ap()
    Wo_d = nc.dram_tensor("Wo", [D, D], F32, kind="ExternalInput").ap()
    bo_d = nc.dram_tensor("bo", [1, D], F32, kind="ExternalInput").ap()
    ones_d = nc.dram_tensor("ones", [1, 128], F32, kind="ExternalInput").ap()
    ind_d = nc.dram_tensor("ind", [128, 8, 8], F32, kind="ExternalInput").ap()
    out_d = nc.dram_tensor("out", [1, D], F32, kind="ExternalOutput").ap()

    with tile.TileContext(nc) as tc, ExitStack() as ctx:
        singles = ctx.enter_context(tc.tile_pool(name="singles", bufs=1))
        vstream = ctx.enter_context(tc.tile_pool(name="vstream", bufs=2))
        vppool = ctx.enter_context(tc.tile_pool(name="vppool", bufs=2))
        expap = ctx.enter_context(tc.tile_pool(name="expap", bufs=2))
        e2pool = ctx.enter_context(tc.tile_pool(name="e2pool", bufs=2))
        smallp = ctx.enter_context(tc.tile_pool(name="smalls", bufs=2))
        rowp = ctx.enter_context(tc.tile_pool(name="rowp", bufs=1))
        vp_ps = ctx.enter_context(tc.tile_pool(name="vp_ps", bufs=2, space="PSUM"))
        aR_ps = ctx.enter_context(tc.tile_pool(name="aR_ps", bufs=2, space="PSUM"))
        s1_ps = ctx.enter_context(tc.tile_pool(name="s1_ps", bufs=1, space="PSUM"))

        # ---- constants / weights ----
        Wv_sb = singles.tile([128, KD, D], F32)       # ktile k at [:, k, :]
        nc.sync.dma_start(out=r(Wv_sb), in_=r(Wv_d.rearrange("(k p) c -> p k c", p=128)))
        iT_sb = singles.tile([128, 8, 128], F32)      # head h: [64*(h%2):+64, h//2, :]
        nc.sync.dma_start(out=r(iT_sb), in_=r(iT_d))
        iN_sb = singles.tile([128, H, DK], BF16)      # i_h as (q, d) at [:, h, :]
        nc.sync.dma_start(out=iN_sb, in_=iN_d)
        cb_sb = singles.tile([128, H], F32)
        nc.sync.dma_start(out=cb_sb, in_=cb_d)
        bo_sb = singles.tile([1, D], F32)
        nc.sync.dma_start(out=r(bo_sb), in_=r(bo_d))
        ones_row = singles.tile([1, 128], F32)        # lhsT for replicate (K=1,M=128)
        nc.sync.dma_start(out=r(ones_row), in_=r(ones_d))
        ind8 = singles.tile([128, 8, 8], F32)         # indicator lhsT per head-in-half
        nc.sync.dma_start(out=r(ind8), in_=r(ind_d))
        Pfold = singles.tile([128, H, LQ], F32)       # per-head fold accumulators

        # ---- main loop over 1024-col chunks ----
        for scp in range(SCP):
            vpT = []  # two (128, KD, 512) SBUF tiles, one per 512 substep
            for sub in range(SUB):
                sidx = scp * SUB + sub
                vT_sb = vstream.tile([128, KD, NCH], F32, tag="vT")
                nc.sync.dma_start(
                    out=r(vT_sb),
                    in_=r(vT_d[:, sidx * NCH:(sidx + 1) * NCH].rearrange(
                        "(k p) s -> p k s", p=128)),
                )
                vp_sb = vppool.tile([128, KD, NCH], F32, tag="vp")
                for cb in range(KD):
                    vp_p = vp_ps.tile([128, NCH], F32, tag="vp_p")
                    for k in range(KD):
                        nc.tensor.matmul(
                            vp_p,
                            lhsT=r(Wv_sb[:, k, cb * 128:(cb + 1) * 128]),
                            rhs=r(vT_sb[:, k, :]),
                            start=(k == 0), stop=(k == KD - 1),
                        )
                    nc.vector.tensor_copy(r(vp_sb[:, cb, :]), r(vp_p))
                vpT.append(vp_sb)

            for hb in range(2):
                s1_p = s1_ps.tile([8, SUB, NCH], F32, tag="s1")
                expq = []  # two (128, 4, 1024) expA quarter tiles
                for qb in range(2):
                    expA = expap.tile([128, 4, SUB * NCH], F32, tag="expa")
                    expq.append(expA)
                    for hi4 in range(4):
                        h = hb * 8 + qb * 4 + hi4
                        hi8 = qb * 4 + hi4
                        po = 64 * (h % 2)
                        a_p = aR_ps.tile([128, SUB, NCH], F32, tag="aR")
                        for sub in range(SUB):
                            nc.tensor.matmul(
                                a_p[:, sub, :],
                                lhsT=r(iT_sb[po:po + 64, h // 2, :]),
                                rhs=r(vpT[sub][po:po + 64, h // 2, :]),
                                start=True, stop=True,
                            )
                        nc.scalar.activation(r(expA[:, hi4, :]), a_p, EXP,
                                             bias=cb_sb[:, h:h + 1])
                        for sub in range(SUB):
                            nc.tensor.matmul(
                                s1_p[:, sub, :],
                                lhsT=r(ind8[:, hi8, :]),
                                rhs=r(expA[:, hi4, sub * NCH:(sub + 1) * NCH]),
                                start=(hi8 == 0), stop=(hi8 == 7),
                            )
                # 1/S1 for the half: copy to SBUF, spread the 8 rows over
                # all 128 partitions via DMA, reciprocal on DVE (avoids ACT
                # Ln/Exp table switching)
                s1_sb = smallp.tile([8, SUB * NCH], F32, tag="s1sb")
                nc.scalar.activation(s1_sb, s1_p,
                                     mybir.ActivationFunctionType.Copy)
                s1_sq = smallp.tile([128, 64], F32, tag="s1sq")
                nc.sync.dma_start(out=s1_sq, in_=s1_sb)
                rsq = smallp.tile([128, 64], F32, tag="rsq")
                nc.vector.reciprocal(rsq, s1_sq)
                for qb in range(2):
                    expA = expq[qb]
                    # PE matmul operands must share base partition 0: flatten
                    # this quarter's 1/S1 rows into a partition-0 row tile
                    rS1r = rowp.tile([1, 4, SUB * NCH], F32, tag="rS1r")
                    nc.sync.dma_start(
                        out=r(rS1r), in_=r(rsq[qb * 64:(qb + 1) * 64, :]))
                    exp2 = e2pool.tile([128, 4, SUB * NCH], BF16, tag="exp2")
                    for hi4 in range(4):
                        h = hb * 8 + qb * 4 + hi4
                        hi8 = qb * 4 + hi4
                        R_p = aR_ps.tile([128, SUB, NCH], F32, tag="aR")
                        for sub in range(SUB):
                            nc.tensor.matmul(
                                R_p[:, sub, :],
                                lhsT=r(ones_row),
                                rhs=r(rS1r[0:1, hi4, sub * NCH:(sub + 1) * NCH]),
                                start=True, stop=True,
                            )
                        # P1 = expA * (1/S1), in place over expA
                        nc.vector.tensor_mul(r(expA[:, hi4, :]), r(expA[:, hi4, :]), R_p)
                    nc.scalar.activation(exp2, expA, EXP, scale=SMOOTH)
                    # fold 8 j-blocks of 128 into Pfold for these 4 heads
                    hlo = hb * 8 + qb * 4
                    e2v = exp2.rearrange("p h (j q) -> p h j q", q=LQ)
                    for j in range(SUB * NCH // LQ):
                        if scp == 0 and j == 0:
                            nc.vector.tensor_copy(Pfold[:, hlo:hlo + 4, :],
                                                  e2v[:, :, j, :])
                        else:
                            nc.vector.tensor_add(Pfold[:, hlo:hlo + 4, :],
                                                 Pfold[:, hlo:hlo + 4, :],
                                                 e2v[:, :, j, :])

        # ---- epilogue ----
        x_p = vp_ps.tile([128, 8], F32, tag="vp_p")
        for h in range(H):
            z2 = smallp.tile([128, 1], F32, tag="z2")
            nc.vector.reduce_sum(z2, Pfold[:, h, :], axis=AX)
            rz2 = smallp.tile([128, 1], F32, tag="rz2")
            nc.vector.reciprocal(rz2, z2)
            pfr = smallp.tile([128, LQ], BF16, tag="pfr")
            nc.vector.tensor_copy(pfr, Pfold[:, h, :])
            rz2r = smallp.tile([128, 1], BF16, tag="rz2r")
            nc.vector.tensor_copy(rz2r, rz2)
            w_p = vp_ps.tile([128, 1], F32, tag="vp_p")
            nc.tensor.matmul(w_p, lhsT=pfr, rhs=rz2r,
                             start=True, stop=True)
            wT = smallp.tile([128, 1], BF16, tag="wT")
            nc.vector.tensor_copy(wT, w_p)
            po = 64 * (h % 2)
            nc.tensor.matmul(
                x_p[po:po + 64, h // 2:h // 2 + 1],
                lhsT=iN_sb[:, h, :],
                rhs=wT,
                start=True, stop=True, skip_group_check=True,
            )
        x_sb = singles.tile([128, 8], F32, tag="x_sb")
        nc.vector.tensor_copy(r(x_sb), r(x_p))
        out_sb = singles.tile([1, D], F32, tag="out_sb")
        for nb in range(2):
            # Wo column-half, reusing the vT stream slots
            Wo_sb = vstream.tile([128, KD, NCH], F32, tag="vT")
            nc.sync.dma_start(
                out=r(Wo_sb),
                in_=r(Wo_d[:, nb * NCH:(nb + 1) * NCH].rearrange(
                    "(k p) c -> p k c", p=128)),
            )
            o_p = aR_ps.tile([1, NCH], F32, tag="aR")
            for j in range(KD):
                nc.tensor.matmul(
                    o_p,
                    lhsT=r(x_sb[:, j:j + 1]),
                    rhs=r(Wo_sb[:, j, :]),
                    start=(j == 0), stop=False,
                )
            nc.tensor.matmul(
                o_p,
                lhsT=r(ones_row[0:1, 0:1]),
                rhs=r(bo_sb[:, nb * NCH:(nb + 1) * NCH]),
                start=False, stop=True,
            )
            nc.vector.tensor_copy(out_sb[:, nb * NCH:(nb + 1) * NCH], o_p)
        nc.sync.dma_start(out=out_d, in_=out_sb)

    nc.compile()
    return nc


def make_in_maps(v, i, Wv, bv, Wo, bo):
    """Shard + lay out inputs per core (core c = batch c)."""
    v = np.ascontiguousarray(np.asarray(v, np.float32))
    i = np.ascontiguousarray(np.asarray(i, np.float32))
    Wv = np.ascontiguousarray(np.asarray(Wv, np.float32))
    Wo = np.ascontiguousarray(np.asarray(Wo, np.float32))
    bv = np.asarray(bv, np.float32)
    bo = np.ascontiguousarray(np.asarray(bo, np.float32)).reshape(1, D)
    in_maps = []
    for b in range(B):
        hv = i[b * H:(b + 1) * H]                      # (16, 128, 64)
        iT = np.zeros((128, 8, 128), np.float32)
        for h in range(H):
            iT[64 * (h % 2):64 * (h % 2) + 64, h // 2, :] = hv[h].T
        import ml_dtypes
        iN = np.ascontiguousarray(np.transpose(hv, (1, 0, 2))).astype(
            ml_dtypes.bfloat16)                                  # (128, 16, 64)
        # C shifts softmax1 logits (exact for softmax; keeps exp/ln in a
        # well-conditioned range for the ACT splines)
        cbm = (np.einsum("hqd,hd->qh", hv, bv.reshape(H, DK))
               - 28.0).astype(np.float32)                       # (128, 16)
        ind = np.zeros((128, 8, 8), np.float32)
        for hi in range(8):
            ind[:, hi, hi] = 1.0
        in_maps.append({
            "ones": np.ones((1, 128), np.float32),
            "ind": ind,
            "vT": np.ascontiguousarray(v[b].T),
            "iT": iT,
            "iN": iN,
            "cb": np.ascontiguousarray(cbm),
            "Wv": Wv,
            "Wo": Wo,
            "bo": bo,
        })
    return in_maps


_NC_CACHE = None


def kernel(q, k, v, i, Wq, bq, Wk, bk, Wv, bv, Wo, bo):
    global _NC_CACHE
    if _NC_CACHE is None:
        _NC_CACHE = build_program()
    nc = _NC_CACHE
    in_maps = make_in_maps(v, i, Wv, bv, Wo, bo)
    res = run_bass_kernel_spmd(nc, in_maps, list(range(8)))
    rows = [res.results[c]["out"].reshape(1, D) for c in range(B)]
    return np.stack(rows, axis=0).astype(np.float32)  # (8, 1, 1024)


if __name__ == "__main__":
    build_program()
    print("compiled OK")



# revision 2
# speedup vs baseline: 1.0088x; 1.0088x over previous
"""Trainium2 Bass kernel for nn_MultiHeadAttention_47330539602717.

Math (per batch b, head h; q/k projections and their biases are dead code):
    vp    = v_b @ Wv                         (4096 s, 1024)   [bv folded separately]
    A_h   = vp_h @ i_h^T                     (4096 s, 128 q)
    P1    = exp(A) / rowsum_q(exp(A))        softmax over q   (free dim!)
    P2    = exp(9 * P1)
    Pfold[qr, l] = sum_j P2[l, qr + 128 j]   (torch raw .view fold over s-tiles)
    Z2[l] = sum_qr Pfold[qr, l]
    w[q]  = sum_l Pfold[q, l] / Z2[l]
    x_h   = w @ i_h                          (64,)
    out_b = concat_h(x_h) @ Wo + bo          (1, 1024)

v2 layout change vs v1: A is computed with s on PARTITIONS and q on the free
dim (A[s, q] rather than A^T[q, s]).  Softmax-1 over q becomes a free-dim
reduction; the 1/S1 scale becomes a per-(partition,head) broadcast multiply
(duplicated-scalar trick keeps DVE in 2x mode); the torch-view fold becomes
plain tile adds; the PE replicate/indicator matmuls of v1 disappear.
Everything post-A runs in bf16/fp16 (validated: rel err ~4.7e-3 vs 2e-2 gate).

Sharding: data-parallel over batch.  Core c handles batch b=c (all 16 heads).

Engine plan per core:
  PE   : vp GEMM (bf16, 512-free MMs), A matmuls (bf16, block-diag i rhs,
         256-free), Pfold transposes + tiny epilogue matmuls
  ACT  : exp1 (PSUM->SBUF, free 1024), exp2 (free 2048), half the vp
         PSUM->SBUF evacuation copies
  DVE  : S1 tree levels 2-3, reciprocal, P1 scale-mult (2x), folds,
         half the vp copies, epilogue reductions
  Pool : S1 tree level 1 (halves-add of E), x9 dup-casts
  DMA  : all inputs bf16 (~12.5 MB/core)
"""

import sys

import numpy as np

sys.path.insert(0, "/opt/trn_rl_repo")

from contextlib import ExitStack

import concourse.bacc as bacc
import concourse.tile as tile
from concourse import mybir
from concourse.bass_utils import run_bass_kernel_spmd

F32 = mybir.dt.float32
BF16 = mybir.dt.bfloat16
FP16 = mybir.dt.float16
EXP = mybir.ActivationFunctionType.Exp
AX = mybir.AxisListType.X

B, LQ, S, D, H = 8, 128, 4096, 1024, 16
DK = D // H          # 64
KD = 8               # k blocks of 128 over D
NG = 4               # s groups
SG = 1024            # s per group
NT = 8               # 128-row s-tiles per group
SMOOTH = 9.0
C1 = -28.0           # exp1 logit shift (exact for softmax, keeps exp in range)
C2 = -4.5            # exp2 shift (cancels in Pfold/Z2; keeps fp16 in range)


def build_program(with_bias=False):
    nc = bacc.Bacc("TRN2", target_bir_lowering=False, debug=False,
                   num_devices=8)

    vT_d = nc.dram_tensor("vT", [128, NG, KD, SG], BF16, kind="ExternalInput").ap()
    Wv_d = nc.dram_tensor("Wv", [128, KD, KD, 128], BF16, kind="ExternalInput").ap()
    ibd_d = nc.dram_tensor("ibd", [128, KD, 256], BF16, kind="ExternalInput").ap()
    iN_d = nc.dram_tensor("iN", [128, H, DK], BF16, kind="ExternalInput").ap()
    idt_d = nc.dram_tensor("idt", [128, 128], BF16, kind="ExternalInput").ap()
    Wo_d = nc.dram_tensor("Wo", [128, KD, D], BF16, kind="ExternalInput").ap()
    bo_d = nc.dram_tensor("bo", [1, D], F32, kind="ExternalInput").ap()
    if with_bias:
        ebv_d = nc.dram_tensor("ebv", [128, H, LQ], BF16, kind="ExternalInput").ap()
    out_d = nc.dram_tensor("out", [1, D], F32, kind="ExternalOutput").ap()

    with tile.TileContext(nc) as tc, ExitStack() as ctx:
        singles = ctx.enter_context(tc.tile_pool(name="singles", bufs=1))
        epool = ctx.enter_context(tc.tile_pool(name="epool", bufs=4))
        spool = ctx.enter_context(tc.tile_pool(name="spool", bufs=4))
        ppool = ctx.enter_context(tc.tile_pool(name="ppool", bufs=3))
        p2pool = ctx.enter_context(tc.tile_pool(name="p2pool", bufs=3))
        vp_ps = ctx.enter_context(tc.tile_pool(name="vp_ps", bufs=2, space="PSUM"))
        a_ps = ctx.enter_context(tc.tile_pool(name="a_ps", bufs=2, space="PSUM"))

        # ---- constants / weights ----
        # DMA priority: vT group 0 + Wv cb-blocks first (they gate the
        # first vp matmuls), then the small attention-side tensors
        vTg = []
        for g in range(NG):
            t = singles.tile([128, KD, SG], BF16, tag=f"vT{g}", name=f"vT{g}")
            vTg.append(t)
        nc.sync.dma_start(out=vTg[0][:, :, 0:512], in_=vT_d[:, 0, :, 0:512])
        nc.sync.dma_start(out=vTg[0][:, :, 512:1024], in_=vT_d[:, 0, :, 512:1024])
        # Wv laid out per cb block: Wv_sb[p, cb, k, c] = Wv[128k+p, 128cb+c]
        Wv_sb = singles.tile([128, KD, KD, 128], BF16, tag="Wv")
        for cb in range(KD):
            nc.sync.dma_start(out=Wv_sb[:, cb], in_=Wv_d[:, cb])
        ibd_sb = singles.tile([128, KD, 256], BF16, tag="ibd")
        nc.sync.dma_start(out=ibd_sb, in_=ibd_d)
        iN_sb = singles.tile([128, H, DK], BF16, tag="iN")
        nc.sync.dma_start(out=iN_sb, in_=iN_d)
        idt_sb = singles.tile([128, 128], BF16, tag="idt")
        nc.sync.dma_start(out=idt_sb, in_=idt_d)
        bo_sb = singles.tile([1, D], F32, tag="bo")
        nc.sync.dma_start(out=bo_sb, in_=bo_d)
        if with_bias:
            ebv_sb = singles.tile([128, H, LQ], BF16, tag="ebv")
            nc.sync.dma_start(out=ebv_sb, in_=ebv_d)
        for g in range(1, NG):
            nc.sync.dma_start(out=vTg[g], in_=vT_d[:, g])
        Wo_sb = singles.tile([128, KD, D], BF16, tag="Wo")
        nc.sync.dma_start(out=Wo_sb, in_=Wo_d)
        vpT = [singles.tile([128, KD, SG], BF16, tag=f"vpT{p}", name=f"vpT{p}")
               for p in range(2)]
        PfA = singles.tile([128, H, LQ], FP16, tag="PfA")
        PfB = singles.tile([128, H, LQ], FP16, tag="PfB")
        c1_sb = singles.tile([128, 1], F32, tag="c1")
        nc.vector.memset(c1_sb, C1)
        c2_sb = singles.tile([128, 1], F32, tag="c2")
        nc.vector.memset(c2_sb, C2)

        # ---- main loop over s-groups of 1024 ----
        def vp_cb(g, cb, sh=None):
            """vp matmuls + PSUM->SBUF copy for one 128-col cb block.
            sh=None: full 1024-col group; sh=0/1: one 512-col half."""
            vpd = vpT[g % 2]
            if sh is None:
                pa = vp_ps.tile([128, 2, 512], F32, tag="vpa", name="pa")
                for k in range(KD):
                    lhs = Wv_sb[:, cb, k, :]
                    nc.tensor.matmul(pa[:, 0, :], lhsT=lhs,
                                     rhs=vTg[g][:, k, 0:512],
                                     start=(k == 0), stop=(k == KD - 1),
                                     skip_group_check=True)
                    nc.tensor.matmul(pa[:, 1, :], lhsT=lhs,
                                     rhs=vTg[g][:, k, 512:1024],
                                     start=(k == 0), stop=(k == KD - 1),
                                     skip_group_check=True)
                dst = vpd[:, cb, :]
            else:
                pa = vp_ps.tile([128, 1, 512], F32, tag="vpa", name="pa")
                for k in range(KD):
                    nc.tensor.matmul(pa[:, 0, :], lhsT=Wv_sb[:, cb, k, :],
                                     rhs=vTg[g][:, k, sh * 512:(sh + 1) * 512],
                                     start=(k == 0), stop=(k == KD - 1),
                                     skip_group_check=True)
                dst = vpd[:, cb, sh * 512:(sh + 1) * 512]
            if cb % 2 == 0:
                nc.vector.tensor_copy(dst, pa)
            else:
                nc.scalar.copy(dst, pa)

        # prologue: only the first 512-col half of group 0's projection, so
        # the elementwise engines start ~14us in instead of ~28us
        for cb in range(KD):
            vp_cb(0, cb, 0)
        for g in range(NG):
            vpd = vpT[g % 2]
            # attention per 128-row s-tile, with remaining projection work
            # software-pipelined in between stiles
            for t in range(NT):
                j = g * NT + t
                if g == 0:
                    vp_jobs = ([(0, 2 * t, 1), (0, 2 * t + 1, 1)] if t < 4
                               else [(1, 2 * (t - 4), None),
                                     (1, 2 * (t - 4) + 1, None)])
                elif g + 1 < NG:
                    vp_jobs = [(g + 1, t, None)]
                else:
                    vp_jobs = []
                E = epool.tile([128, H, LQ], BF16, tag="E")
                for half in range(2):
                    ap = a_ps.tile([128, 8, 128], F32, tag="aps")
                    for c4 in range(4):
                        cb = half * 4 + c4
                        nc.tensor.matmul(
                            ap[:, 2 * c4:2 * c4 + 2, :],
                            lhsT=vpd[:, cb, t * 128:(t + 1) * 128],
                            rhs=ibd_sb[:, cb, :],
                            start=True, stop=True, skip_group_check=True,
                        )
                    nc.scalar.activation(E[:, half * 8:(half + 1) * 8, :],
                                         ap, EXP, bias=c1_sb[:, 0:1])
                if with_bias:
                    E2 = epool.tile([128, H, LQ], BF16, tag="E")
                    nc.vector.tensor_mul(E2, E, ebv_sb)
                    E = E2
                # S1: one halves-add level then reduce (TENSOR_REDUCE is
                # slow per element, so shrink its input first)
                T1 = spool.tile([128, H, 64], BF16, tag="T1")
                nc.vector.tensor_add(T1, E[:, :, 0:64], E[:, :, 64:128])
                S1t = spool.tile([128, H], F32, tag="S1t")
                nc.vector.reduce_sum(S1t, T1, axis=AX)
                # 1/S1 duplicated pairwise: a [1,2]-packed last dim keeps the
                # broadcast-mult AP off the slow stride-0-last-dim path
                rd = spool.tile([128, H, 2], F32, tag="rd")
                for dd in range(2):
                    nc.vector.reciprocal(rd[:, :, dd], S1t)
                P = ppool.tile([128, H, LQ], FP16, tag="P")
                Ev = E.rearrange("p h (a b) -> p h a b", b=2)
                Pv = P.rearrange("p h (a b) -> p h a b", b=2)
                rv = rd.unsqueeze(2).broadcast_to([128, H, 64, 2])
                nc.gpsimd.tensor_mul(Pv, Ev, rv)
                # exp2 = exp(9*P1 - 4.5): the x9 rides in the ACT scale
                P2 = p2pool.tile([128, H, LQ], FP16, tag="P2")
                nc.scalar.activation(P2, P, EXP, bias=c2_sb[:, 0:1],
                                     scale=SMOOTH)
                # fold: two DVE accumulator chains (even/odd stiles)
                Pf = PfA if j % 2 == 0 else PfB
                if j < 2:
                    nc.vector.tensor_copy(Pf, P2)
                else:
                    nc.vector.tensor_add(Pf, Pf, P2)
                for (gg, cb, sh) in vp_jobs:
                    vp_cb(gg, cb, sh)

        # ---- epilogue ----
        Pfs = singles.tile([128, H, LQ], BF16, tag="Pfs")
        nc.vector.tensor_add(Pfs, PfA, PfB)
        # transpose per head: Pfold[qr, h, l] -> pT[l, h, qr]
        pT_sb = singles.tile([128, H, LQ], BF16, tag="pT")
        for q8 in range(2):
            tp = a_ps.tile([128, 8, 128], BF16, tag="aps")
            for hh in range(8):
                h = q8 * 8 + hh
                nc.tensor.matmul(tp[:, hh, :], lhsT=Pfs[:, h, :], rhs=idt_sb,
                                 is_transpose=True, start=True, stop=True,
                                 skip_group_check=True)
            nc.vector.tensor_copy(pT_sb[:, q8 * 8:(q8 + 1) * 8, :], tp)
        z2 = singles.tile([128, H], F32, tag="z2")
        nc.vector.reduce_sum(z2, pT_sb, axis=AX)
        rz2 = singles.tile([128, H], F32, tag="rz2")
        nc.vector.reciprocal(rz2, z2)
        rz2b = singles.tile([128, H], BF16, tag="rz2b")
        nc.vector.tensor_copy(rz2b, rz2)
        wp = vp_ps.tile([128, H], F32, tag="vpa")
        for h in range(H):
            nc.tensor.matmul(wp[:, h:h + 1], lhsT=pT_sb[:, h, :],
                             rhs=rz2b[:, h:h + 1],
                             start=True, stop=True, skip_group_check=True)
        wb = singles.tile([128, H], BF16, tag="wb")
        nc.vector.tensor_copy(wb, wp)
        xp = vp_ps.tile([128, KD], F32, tag="vpa")
        for h in range(H):
            po = 64 * (h % 2)
            nc.tensor.matmul(
                xp[po:po + 64, h // 2:h // 2 + 1],
                lhsT=iN_sb[:, h, :], rhs=wb[:, h:h + 1],
                start=True, stop=True, skip_group_check=True)
        xb = singles.tile([128, KD], BF16, tag="xb")
        nc.vector.tensor_copy(xb, xp)
        out_sb = singles.tile([1, D], F32, tag="osb")
        for nb in range(2):
            op = a_ps.tile([1, 512], F32, tag="aps")
            for jj in range(KD):
                nc.tensor.matmul(op, lhsT=xb[:, jj:jj + 1],
                                 rhs=Wo_sb[:, jj, nb * 512:(nb + 1) * 512],
                                 start=(jj == 0), stop=(jj == KD - 1))
            nc.vector.tensor_add(out_sb[:, nb * 512:(nb + 1) * 512], op,
                                 bo_sb[:, nb * 512:(nb + 1) * 512])
            nc.sync.dma_start(out=out_d[:, nb * 512:(nb + 1) * 512],
                              in_=out_sb[:, nb * 512:(nb + 1) * 512])

    nc.compile()
    return nc


def make_in_maps(v, i, Wv, bv, Wo, bo):
    """Shard + lay out inputs per core (core c = batch c), all bf16."""
    import ml_dtypes
    bf16 = ml_dtypes.bfloat16
    v = np.asarray(v, np.float32)
    i = np.asarray(i, np.float32)
    # Wv[p, cb, k, c] = Wv[128k+p, 128cb+c]
    Wv16 = np.ascontiguousarray(
        np.asarray(Wv, np.float32).reshape(KD, 128, KD, 128).transpose(1, 2, 0, 3)
    ).astype(bf16)
    Wo16 = np.ascontiguousarray(
        np.asarray(Wo, np.float32).reshape(KD, 128, D).transpose(1, 0, 2)
    ).astype(bf16)
    bo = np.ascontiguousarray(np.asarray(bo, np.float32)).reshape(1, D)
    bv = np.asarray(bv, np.float32)
    with_bias = bool(np.any(bv))
    idt = np.eye(128, dtype=np.float32).astype(bf16)
    in_maps = []
    for b in range(B):
        hv = i[b * H:(b + 1) * H]                     # (16, 128, 64)
        hv16 = hv.astype(bf16)
        # vT[p, g, k, s'] = v[b][g*1024+s', 128k+p]
        v4 = v[b].reshape(NG, SG, KD, 128)
        vT = np.ascontiguousarray(v4.transpose(3, 0, 2, 1)).astype(bf16)
        ibd = np.zeros((128, KD, 256), bf16)
        for cb in range(KD):
            ibd[0:64, cb, 0:128] = hv16[2 * cb].T
            ibd[64:128, cb, 128:256] = hv16[2 * cb + 1].T
        iN = np.ascontiguousarray(np.transpose(hv16, (1, 0, 2)))  # (128,16,64)
        m = {
            "vT": vT, "Wv": Wv16, "ibd": np.ascontiguousarray(ibd),
            "iN": iN, "idt": idt, "Wo": Wo16, "bo": bo,
        }
        if with_bias:
            ebv = np.exp(np.einsum("hqd,hd->qh", hv, bv.reshape(H, DK)))
            m["ebv"] = np.ascontiguousarray(
                np.repeat(ebv[:, :, None], LQ, axis=2).reshape(128, H, LQ)
            ).astype(bf16)
        in_maps.append(m)
    return in_maps, with_bias


_NC_CACHE = {}


def kernel(q, k, v, i, Wq, bq, Wk, bk, Wv, bv, Wo, bo):
    in_maps, with_bias = make_in_maps(v, i, Wv, bv, Wo, bo)
    key = with_bias
    if key not in _NC_CACHE:
        _NC_CACHE[key] = build_program(with_bias=with_bias)
    nc = _NC_CACHE[key]
    res = run_bass_kernel_spmd(nc, in_maps, list(range(8)))
    rows = [res.results[c]["out"].reshape(1, D) for c in range(B)]
    return np.stack(rows, axis=0).astype(np.float32)  # (8, 1, 1024)


if __name__ == "__main__":
    build_program()
    print("compiled OK")


# revision 3
# speedup vs baseline: 1.0138x; 1.0049x over previous
"""Trainium2 Bass kernel for nn_MultiHeadAttention_47330539602717.

Math (per batch b, head h; q/k projections and their biases are dead code):
    vp    = v_b @ Wv                         (4096 s, 1024)   [bv folded separately]
    A_h   = vp_h @ i_h^T                     (4096 s, 128 q)
    P1    = exp(A) / rowsum_q(exp(A))        softmax over q   (free dim!)
    P2    = exp(9 * P1)
    Pfold[qr, l] = sum_j P2[l, qr + 128 j]   (torch raw .view fold over s-tiles)
    Z2[l] = sum_qr Pfold[qr, l]
    w[q]  = sum_l Pfold[q, l] / Z2[l]
    x_h   = w @ i_h                          (64,)
    out_b = concat_h(x_h) @ Wo + bo          (1, 1024)

v2 layout change vs v1: A is computed with s on PARTITIONS and q on the free
dim (A[s, q] rather than A^T[q, s]).  Softmax-1 over q becomes a free-dim
reduction; the 1/S1 scale becomes a per-(partition,head) broadcast multiply
(duplicated-scalar trick keeps DVE in 2x mode); the torch-view fold becomes
plain tile adds; the PE replicate/indicator matmuls of v1 disappear.
Everything post-A runs in bf16/fp16 (validated: rel err ~4.7e-3 vs 2e-2 gate).

Sharding: data-parallel over batch.  Core c handles batch b=c (all 16 heads).

Engine plan per core:
  PE   : vp GEMM (bf16, 512-free MMs), A matmuls (bf16, block-diag i rhs,
         256-free), Pfold transposes + tiny epilogue matmuls
  ACT  : exp1 (PSUM->SBUF, free 1024), exp2 (free 2048), half the vp
         PSUM->SBUF evacuation copies
  DVE  : S1 tree levels 2-3, reciprocal, P1 scale-mult (2x), folds,
         half the vp copies, epilogue reductions
  Pool : S1 tree level 1 (halves-add of E), x9 dup-casts
  DMA  : all inputs bf16 (~12.5 MB/core)
"""

import sys

import numpy as np

sys.path.insert(0, "/opt/trn_rl_repo")

from contextlib import ExitStack

import concourse.bacc as bacc
import concourse.tile as tile
from concourse import mybir
from concourse.bass_utils import run_bass_kernel_spmd

F32 = mybir.dt.float32
BF16 = mybir.dt.bfloat16
FP16 = mybir.dt.float16
EXP = mybir.ActivationFunctionType.Exp
AX = mybir.AxisListType.X

B, LQ, S, D, H = 8, 128, 4096, 1024, 16
DK = D // H          # 64
KD = 8               # k blocks of 128 over D
NG = 4               # s groups
SG = 1024            # s per group
NT = 8               # 128-row s-tiles per group
SMOOTH = 9.0
C1 = -28.0           # exp1 logit shift (exact for softmax, keeps exp in range)
C2 = -4.5            # exp2 shift (cancels in Pfold/Z2; keeps fp16 in range)


def build_program(with_bias=False):
    nc = bacc.Bacc("TRN2", target_bir_lowering=False, debug=False,
                   num_devices=8)

    vT_d = nc.dram_tensor("vT", [128, NG, KD, SG], BF16, kind="ExternalInput").ap()
    Wv_d = nc.dram_tensor("Wv", [128, KD, KD, 128], BF16, kind="ExternalInput").ap()
    ibd_d = nc.dram_tensor("ibd", [128, KD, 256], BF16, kind="ExternalInput").ap()
    iN_d = nc.dram_tensor("iN", [128, H, DK], BF16, kind="ExternalInput").ap()
    idt_d = nc.dram_tensor("idt", [128, 128], BF16, kind="ExternalInput").ap()
    Wo_d = nc.dram_tensor("Wo", [128, KD, D], BF16, kind="ExternalInput").ap()
    bo_d = nc.dram_tensor("bo", [1, D], F32, kind="ExternalInput").ap()
    if with_bias:
        ebv_d = nc.dram_tensor("ebv", [128, H, LQ], BF16, kind="ExternalInput").ap()
    out_d = nc.dram_tensor("out", [1, D], F32, kind="ExternalOutput").ap()

    with tile.TileContext(nc) as tc, ExitStack() as ctx:
        singles = ctx.enter_context(tc.tile_pool(name="singles", bufs=1))
        epool = ctx.enter_context(tc.tile_pool(name="epool", bufs=4))
        spool = ctx.enter_context(tc.tile_pool(name="spool", bufs=3))
        ppool = ctx.enter_context(tc.tile_pool(name="ppool", bufs=3))
        p2pool = ctx.enter_context(tc.tile_pool(name="p2pool", bufs=3))
        vp_ps = ctx.enter_context(tc.tile_pool(name="vp_ps", bufs=2, space="PSUM"))
        a_ps = ctx.enter_context(tc.tile_pool(name="a_ps", bufs=2, space="PSUM"))

        # ---- constants / weights ----
        # DMA priority: vT group 0 + Wv cb-blocks first (they gate the
        # first vp matmuls), then the small attention-side tensors
        vTg = []
        for g in range(NG):
            t = singles.tile([128, KD, SG], BF16, tag=f"vT{g}", name=f"vT{g}")
            vTg.append(t)
        nc.sync.dma_start(out=vTg[0][:, :, 0:512], in_=vT_d[:, 0, :, 0:512])
        nc.sync.dma_start(out=vTg[0][:, :, 512:1024], in_=vT_d[:, 0, :, 512:1024])
        # Wv laid out per cb block: Wv_sb[p, cb, k, c] = Wv[128k+p, 128cb+c]
        Wv_sb = singles.tile([128, KD, KD, 128], BF16, tag="Wv")
        for cb in range(KD):
            nc.sync.dma_start(out=Wv_sb[:, cb], in_=Wv_d[:, cb])
        ibd_sb = singles.tile([128, KD, 256], BF16, tag="ibd")
        nc.sync.dma_start(out=ibd_sb, in_=ibd_d)
        iN_sb = singles.tile([128, H, DK], BF16, tag="iN")
        nc.sync.dma_start(out=iN_sb, in_=iN_d)
        idt_sb = singles.tile([128, 128], BF16, tag="idt")
        nc.sync.dma_start(out=idt_sb, in_=idt_d)
        bo_sb = singles.tile([1, D], F32, tag="bo")
        nc.sync.dma_start(out=bo_sb, in_=bo_d)
        if with_bias:
            ebv_sb = singles.tile([128, H, LQ], BF16, tag="ebv")
            nc.sync.dma_start(out=ebv_sb, in_=ebv_d)
        for g in range(1, NG):
            nc.sync.dma_start(out=vTg[g], in_=vT_d[:, g])
        Wo_sb = singles.tile([128, KD, D], BF16, tag="Wo")
        nc.sync.dma_start(out=Wo_sb, in_=Wo_d)
        vpT = [singles.tile([128, KD, SG], BF16, tag=f"vpT{p}", name=f"vpT{p}")
               for p in range(2)]
        PfA = singles.tile([128, H, LQ], FP16, tag="PfA")
        PfB = singles.tile([128, H, LQ], FP16, tag="PfB")
        c1_sb = singles.tile([128, 1], F32, tag="c1")
        nc.vector.memset(c1_sb, C1)
        c2_sb = singles.tile([128, 1], F32, tag="c2")
        nc.vector.memset(c2_sb, C2)
        idt16 = singles.tile([128, 128], FP16, tag="idt16")
        nc.vector.tensor_copy(idt16, idt_sb)

        # ---- main loop over s-groups of 1024 ----
        def vp_cb(g, cb, sh=None):
            """vp matmuls + PSUM->SBUF copy for one 128-col cb block.
            sh=None: full 1024-col group; sh=0/1: one 512-col half."""
            vpd = vpT[g % 2]
            if sh is None:
                pa = vp_ps.tile([128, 2, 512], F32, tag="vpa", name="pa")
                for k in range(KD):
                    lhs = Wv_sb[:, cb, k, :]
                    nc.tensor.matmul(pa[:, 0, :], lhsT=lhs,
                                     rhs=vTg[g][:, k, 0:512],
                                     start=(k == 0), stop=(k == KD - 1),
                                     skip_group_check=True)
                    nc.tensor.matmul(pa[:, 1, :], lhsT=lhs,
                                     rhs=vTg[g][:, k, 512:1024],
                                     start=(k == 0), stop=(k == KD - 1),
                                     skip_group_check=True)
                dst = vpd[:, cb, :]
            else:
                pa = vp_ps.tile([128, 1, 512], F32, tag="vpa", name="pa")
                for k in range(KD):
                    nc.tensor.matmul(pa[:, 0, :], lhsT=Wv_sb[:, cb, k, :],
                                     rhs=vTg[g][:, k, sh * 512:(sh + 1) * 512],
                                     start=(k == 0), stop=(k == KD - 1),
                                     skip_group_check=True)
                dst = vpd[:, cb, sh * 512:(sh + 1) * 512]
            if cb % 2 == 0:
                nc.vector.tensor_copy(dst, pa)
            else:
                nc.scalar.copy(dst, pa)

        # prologue: only the first 512-col half of group 0's projection, so
        # the elementwise engines start ~14us in instead of ~28us
        for cb in range(KD):
            vp_cb(0, cb, 0)
        for g in range(NG):
            vpd = vpT[g % 2]
            # attention per 128-row s-tile, with remaining projection work
            # software-pipelined in between stiles
            for t in range(NT):
                j = g * NT + t
                if g == 0:
                    vp_jobs = ([(0, 2 * t, 1), (0, 2 * t + 1, 1)] if t < 4
                               else [(1, 2 * (t - 4), None),
                                     (1, 2 * (t - 4) + 1, None)])
                elif g + 1 < NG:
                    vp_jobs = [(g + 1, t, None)]
                else:
                    vp_jobs = []
                E = epool.tile([128, H, LQ], BF16, tag="E")
                for half in range(2):
                    ap = a_ps.tile([128, 8, 128], F32, tag="aps")
                    for c4 in range(4):
                        cb = half * 4 + c4
                        nc.tensor.matmul(
                            ap[:, 2 * c4:2 * c4 + 2, :],
                            lhsT=vpd[:, cb, t * 128:(t + 1) * 128],
                            rhs=ibd_sb[:, cb, :],
                            start=True, stop=True, skip_group_check=True,
                        )
                    nc.scalar.activation(E[:, half * 8:(half + 1) * 8, :],
                                         ap, EXP, bias=c1_sb[:, 0:1])
                if with_bias:
                    E2 = epool.tile([128, H, LQ], BF16, tag="E")
                    nc.vector.tensor_mul(E2, E, ebv_sb)
                    E = E2
                # S1 reduced straight from E: slightly more DVE engine time
                # than a halves-add tree, but 4KB/partition less SBUF-port
                # traffic per stile (DVE+GpSimd are port-co-limited)
                S1t = spool.tile([128, H], F32, tag="S1t")
                nc.vector.reduce_sum(S1t, E, axis=AX)
                # 1/S1 duplicated pairwise: a [1,2]-packed last dim keeps the
                # broadcast-mult AP off the slow stride-0-last-dim path
                rd = spool.tile([128, H, 2], F32, tag="rd")
                nc.vector.reciprocal(
                    rd, S1t.unsqueeze(2).broadcast_to([128, H, 2]))
                P = ppool.tile([128, H, LQ], FP16, tag="P")
                Ev = E.rearrange("p h (a b) -> p h a b", b=2)
                Pv = P.rearrange("p h (a b) -> p h a b", b=2)
                rv = rd.unsqueeze(2).broadcast_to([128, H, 64, 2])
                nc.gpsimd.tensor_mul(Pv, Ev, rv)
                # exp2 = exp(9*P1 - 4.5): the x9 rides in the ACT scale
                P2 = p2pool.tile([128, H, LQ], FP16, tag="P2")
                nc.scalar.activation(P2, P, EXP, bias=c2_sb[:, 0:1],
                                     scale=SMOOTH)
                # fold: two DVE accumulator chains (even/odd stiles)
                Pf = PfA if j % 2 == 0 else PfB
                if j < 2:
                    nc.vector.tensor_copy(Pf, P2)
                else:
                    nc.vector.tensor_add(Pf, Pf, P2)
                for (gg, cb, sh) in vp_jobs:
                    vp_cb(gg, cb, sh)

        # ---- epilogue ----
        Pfs = singles.tile([128, H, LQ], BF16, tag="Pfs")
        nc.vector.tensor_add(Pfs, PfA, PfB)
        # transpose per head: Pfold[qr, h, l] -> pT[l, h, qr]
        pT_sb = singles.tile([128, H, LQ], BF16, tag="pT")
        for q8 in range(2):
            tp = a_ps.tile([128, 8, 128], BF16, tag="aps")
            for hh in range(8):
                h = q8 * 8 + hh
                nc.tensor.matmul(tp[:, hh, :], lhsT=Pfs[:, h, :], rhs=idt_sb,
                                 is_transpose=True, start=True, stop=True,
                                 skip_group_check=True)
            nc.vector.tensor_copy(pT_sb[:, q8 * 8:(q8 + 1) * 8, :], tp)
        z2 = singles.tile([128, H], F32, tag="z2")
        nc.vector.reduce_sum(z2, pT_sb, axis=AX)
        rz2 = singles.tile([128, H], F32, tag="rz2")
        nc.vector.reciprocal(rz2, z2)
        rz2b = singles.tile([128, H], BF16, tag="rz2b")
        nc.vector.tensor_copy(rz2b, rz2)
        wp = vp_ps.tile([128, H], F32, tag="vpa")
        for h in range(H):
            nc.tensor.matmul(wp[:, h:h + 1], lhsT=pT_sb[:, h, :],
                             rhs=rz2b[:, h:h + 1],
                             start=True, stop=True, skip_group_check=True)
        wb = singles.tile([128, H], BF16, tag="wb")
        nc.vector.tensor_copy(wb, wp)
        xp = vp_ps.tile([128, KD], F32, tag="vpa")
        for h in range(H):
            po = 64 * (h % 2)
            nc.tensor.matmul(
                xp[po:po + 64, h // 2:h // 2 + 1],
                lhsT=iN_sb[:, h, :], rhs=wb[:, h:h + 1],
                start=True, stop=True, skip_group_check=True)
        xb = singles.tile([128, KD], BF16, tag="xb")
        nc.vector.tensor_copy(xb, xp)
        out_sb = singles.tile([1, D], F32, tag="osb")
        for nb in range(2):
            op = a_ps.tile([1, 512], F32, tag="aps")
            for jj in range(KD):
                nc.tensor.matmul(op, lhsT=xb[:, jj:jj + 1],
                                 rhs=Wo_sb[:, jj, nb * 512:(nb + 1) * 512],
                                 start=(jj == 0), stop=(jj == KD - 1))
            nc.vector.tensor_add(out_sb[:, nb * 512:(nb + 1) * 512], op,
                                 bo_sb[:, nb * 512:(nb + 1) * 512])
            nc.sync.dma_start(out=out_d[:, nb * 512:(nb + 1) * 512],
                              in_=out_sb[:, nb * 512:(nb + 1) * 512])

    nc.compile()
    return nc


def make_in_maps(v, i, Wv, bv, Wo, bo):
    """Shard + lay out inputs per core (core c = batch c), all bf16."""
    import ml_dtypes
    bf16 = ml_dtypes.bfloat16
    v = np.asarray(v, np.float32)
    i = np.asarray(i, np.float32)
    # Wv[p, cb, k, c] = Wv[128k+p, 128cb+c]
    Wv16 = np.ascontiguousarray(
        np.asarray(Wv, np.float32).reshape(KD, 128, KD, 128).transpose(1, 2, 0, 3)
    ).astype(bf16)
    Wo16 = np.ascontiguousarray(
        np.asarray(Wo, np.float32).reshape(KD, 128, D).transpose(1, 0, 2)
    ).astype(bf16)
    bo = np.ascontiguousarray(np.asarray(bo, np.float32)).reshape(1, D)
    bv = np.asarray(bv, np.float32)
    with_bias = bool(np.any(bv))
    idt = np.eye(128, dtype=np.float32).astype(bf16)
    in_maps = []
    for b in range(B):
        hv = i[b * H:(b + 1) * H]                     # (16, 128, 64)
        hv16 = hv.astype(bf16)
        # vT[p, g, k, s'] = v[b][g*1024+s', 128k+p]
        v4 = v[b].reshape(NG, SG, KD, 128)
        vT = np.ascontiguousarray(v4.transpose(3, 0, 2, 1)).astype(bf16)
        ibd = np.zeros((128, KD, 256), bf16)
        for cb in range(KD):
            ibd[0:64, cb, 0:128] = hv16[2 * cb].T
            ibd[64:128, cb, 128:256] = hv16[2 * cb + 1].T
        iN = np.ascontiguousarray(np.transpose(hv16, (1, 0, 2)))  # (128,16,64)
        m = {
            "vT": vT, "Wv": Wv16, "ibd": np.ascontiguousarray(ibd),
            "iN": iN, "idt": idt, "Wo": Wo16, "bo": bo,
        }
        if with_bias:
            ebv = np.exp(np.einsum("hqd,hd->qh", hv, bv.reshape(H, DK)))
            m["ebv"] = np.ascontiguousarray(
                np.repeat(ebv[:, :, None], LQ, axis=2).reshape(128, H, LQ)
            ).astype(bf16)
        in_maps.append(m)
    return in_maps, with_bias


_NC_CACHE = {}


def kernel(q, k, v, i, Wq, bq, Wk, bk, Wv, bv, Wo, bo):
    in_maps, with_bias = make_in_maps(v, i, Wv, bv, Wo, bo)
    key = with_bias
    if key not in _NC_CACHE:
        _NC_CACHE[key] = build_program(with_bias=with_bias)
    nc = _NC_CACHE[key]
    res = run_bass_kernel_spmd(nc, in_maps, list(range(8)))
    rows = [res.results[c]["out"].reshape(1, D) for c in range(B)]
    return np.stack(rows, axis=0).astype(np.float32)  # (8, 1, 1024)


if __name__ == "__main__":
    build_program()
    print("compiled OK")
